# revision 58
# baseline (speedup 1.0000x reference)
"""Causal self-attention on 8 TRN2 NeuronCores.

Sharding: 8 cores = 4 batches x 2 head-groups (data parallel on B,
tensor parallel on heads). Core (b, g) computes batch b, heads
8g..8g+7 end-to-end (qkv slice -> causal attention -> partial
projection); the host sums the two per-batch partials.

All-bf16 dataflow: q/k/v/O stay SBUF-resident (no DRAM round-trips),
matmuls run bf16 (same PE rate as fp32r, but FWL halves the stationary
load and all DMA/SBUF footprints halve). Attention runs as one flat
software pipeline across all heads (pair-of-k-chunks slots; PV/r one
slot behind S/exp so the PE never waits on the exp chain). Diagonal
chunks are trimmed to their valid q-suffix, with only the 128x128
leading triangle masked (multiplicative 0/1 on DVE after exp). The
softmax denominator is PSUM-accumulated by ones-matmuls riding with
PV; 1/r uses the fast DVE reciprocal and a DMA-bounce broadcast.

Self-contained: hardcodes B=4, T=2048, C=2048, H=16, HD=128.
"""

import numpy as np

import concourse.bass as bass
import concourse.mybir as mybir
import concourse.tile as tile
from concourse import bacc
from concourse.bass_utils import run_bass_kernel_spmd

B, T, C, H = 4, 2048, 2048, 16
HD = 128          # head dim
G = 2             # head groups (tensor parallel)
HPG = H // G      # 8 heads per core
DG = HPG * HD     # 1024 = per-core concat head dim
N_CORES = 8
SCALE = float(HD) ** -0.5
NEG = -1.0e9      # additive causal mask value

F32 = mybir.dt.float32
F32R = mybir.dt.float32r
BF16 = mybir.dt.bfloat16

P = 128           # partitions
FN = 512          # moving free-dim per matmul (one PSUM bank of fp32)
CI = C // P       # 16 contraction chunks over C
TM = T // P       # 16 t chunks of 128
TN = T // FN      # 4 t chunks of 512
MQK = 2 * DG // P # 16 qk^T row chunks
NV = DG // FN     # 2 v column chunks of 512
HC = CI // 2      # wv streamed in half-ci tiles


def build_nc() -> bass.Bass:
    nc = bacc.Bacc()
    # xt = x.T (host pre-transposed, bf16). wqk = [wq|wk] cols for this
    # core's heads, host-permuted to [m, p, ci*128+col]. wv = v cols,
    # host-permuted to [nv, p, ci*512+vc]. wp = w_proj rows, host-
    # permuted to [p, hh, n]. All weights bf16.
    xt = nc.declare_dram_parameter("xt", [C, T], BF16, isOutput=False)
    wqk = nc.declare_dram_parameter("wqk", [MQK, P, CI * P], BF16, isOutput=False)
    wv = nc.declare_dram_parameter("wv", [NV, P, CI * FN], BF16, isOutput=False)
    wp = nc.declare_dram_parameter("wp", [P, HPG, C], BF16, isOutput=False)
    masks = nc.declare_dram_parameter("masks", [P, P], BF16, isOutput=False)
    out = nc.declare_dram_parameter("out", [T, C], F32, isOutput=True)

    with tile.TileContext(nc) as tc:
        with (
            tc.tile_pool(name="consts", bufs=1) as consts,
            tc.tile_pool(name="dram", bufs=1, space="DRAM") as dram_pool,
        ):
            ones_f = consts.tile([P, 1], F32)
            nc.gpsimd.memset(ones_f[:], 1.0)
            ones_bf = consts.tile([P, 1], BF16)
            nc.scalar.copy(ones_bf[:], ones_f[:])
            masks_sb = consts.tile([P, P], BF16)  # 0/1 diagonal triangle
            nc.sync.dma_start(masks_sb[:], masks[:, :])
            # warm the exp activation table while ACT is idle
            warm = consts.tile([1, 1], F32)
            nc.scalar.activation(
                warm[:], ones_f[0:1, :], mybir.ActivationFunctionType.Exp
            )

            with (
                tc.tile_pool(name="qkT", bufs=1) as qkT_pool,
                tc.tile_pool(name="vn", bufs=1) as vn_pool,
            ):
                # qk^T resident: m 0..7 = q^T per head, 8..15 = k^T per head
                qkT = [qkT_pool.tile([P, T], BF16, name=f"qkT{m}") for m in range(MQK)]
                # V natural chunks: [t-within-chunk, head, tm, d]
                vn = vn_pool.tile([P, HPG, TM, HD], BF16)

                with tc.tile_pool(name="xT", bufs=1) as xT_pool:
                    xT = xT_pool.tile([P, CI, T], BF16)  # x^T resident 64KB/part

                    # ---------- Phase A: qk^T = wqk.T @ x.T ----------
                    # nt-outer: each nt sweep needs only its 2MB of x^T, so
                    # the PE starts a few us in. wq is re-streamed per nt
                    # (4x wqk traffic, all hidden under compute).
                    QC = 4  # ci per wv tile
                    with (
                        tc.tile_pool(name="wq", bufs=3) as wq_pool,
                        tc.tile_pool(name="aps", bufs=4, space="PSUM") as apsum,
                        tc.tile_pool(name="wvp", bufs=1) as wv_pool,
                        tc.tile_pool(name="bps", bufs=4, space="PSUM") as bpsum,
                    ):
                        wq_pre = []
                        for m in range(2):  # first wq tiles ahead of the x
                            w = wq_pool.tile([P, CI, P], BF16, tag="wq")  # DMAs;
                            for ci in range(CI):  # per-ci so matmul 0 waits
                                nc.sync.dma_start(  # on 32KB, not 512KB
                                    w[:, ci, :], wqk[m, :, ci * P:(ci + 1) * P]
                                )
                            wq_pre.append(w)
                        wvt = {}
                        for nt in range(TN):
                            t0, t1 = nt * FN, (nt + 1) * FN
                            for ci in range(CI):
                                nc.sync.dma_start(
                                    xT[:, ci, t0:t1], xt[ci * P:(ci + 1) * P, t0:t1]
                                )
                            for m in range(MQK):
                                if nt == 0 and m < 2:
                                    wq = wq_pre[m]
                                else:
                                    wq = wq_pool.tile([P, CI, P], BF16, tag="wq")
                                    nc.sync.dma_start(
                                        wq[:],
                                        wqk[m, :, :]
                                        .rearrange("p (ci n) -> p ci n", ci=CI),
                                    )
                                ps = apsum.tile([P, FN], F32)
                                for ci in range(CI):
                                    nc.tensor.matmul(
                                        ps[:], wq[:, ci, :], xT[:, ci, t0:t1],
                                        start=(ci == 0), stop=(ci == CI - 1),
                                    )
                                if m % 2 == 0:
                                    nc.vector.tensor_copy(qkT[m][:, t0:t1], ps[:])
                                else:
                                    nc.scalar.copy(qkT[m][:, t0:t1], ps[:])
                            if nt == 0:
                                # wv resident loads ride behind nt 0's data,
                                # arriving long before phase B
                                for nv in range(NV):
                                    for q in range(CI // QC):
                                        t = wv_pool.tile(
                                            [P, QC, FN], BF16, name=f"wvt{nv}_{q}"
                                        )
                                        nc.sync.dma_start(
                                            t[:],
                                            wv[nv, :, q * QC * FN:(q + 1) * QC * FN]
                                            .rearrange("p (ci n) -> p ci n", ci=QC),
                                        )
                                        wvt[nv, q] = t

                        # ---------- Phase B: V = x @ wv ----------
                        for nv in range(NV):
                            for tm in range(TM):
                                ps = bpsum.tile([P, FN], F32)
                                for ci in range(CI):
                                    nc.tensor.matmul(
                                        ps[:], xT[:, ci, tm * P:(tm + 1) * P],
                                        wvt[nv, ci // QC][:, ci % QC, :],
                                        start=(ci == 0), stop=(ci == CI - 1),
                                    )
                                # DVE-only near the end so ACT is free for
                                # phase C's first exps
                                if tm % 2 == 0 or (nv == NV - 1 and tm >= TM - 4):
                                    nc.vector.tensor_copy(ps_dest_vn(vn, nv, tm), ps[:])
                                else:
                                    nc.scalar.copy(ps_dest_vn(vn, nv, tm), ps[:])

                # xT freed; wpt + oT live in its space through C..D
                with (
                    tc.tile_pool(name="wp", bufs=1) as wp_pool,
                    tc.tile_pool(name="oTp", bufs=1) as oT_pool,
                ):
                    wpt = wp_pool.tile([P, HPG, C], BF16)
                    for hh in range(HPG):
                        nc.sync.dma_start(wpt[:, hh, :], wp[:, hh, :])
                    oT = oT_pool.tile([P, HPG, T], BF16)  # attention out^T

                    # ---------- Phase C: causal attention per head ----------
                    with (
                        tc.tile_pool(name="cps", bufs=2, space="PSUM") as cps,
                        tc.tile_pool(name="ops", bufs=2, space="PSUM") as ops,
                        tc.tile_pool(name="prs", bufs=2, space="PSUM") as prs,
                        tc.tile_pool(name="pt", bufs=5) as pt_pool,
                        tc.tile_pool(name="rr", bufs=3) as rr_pool,
                    ):
                        # one flat pipeline over all heads: slot = (h, j, p)
                        # with 2 k-chunks per slot; no per-head drain.
                        # Diagonal chunks (i >= 4j) are trimmed: the fully-
                        # invalid q-prefix of width 128*(i-4j) is skipped in
                        # S/PV/r (the stale ptt there is never read), and only
                        # the 128x128 leading triangle needs a 0/1 mask.
                        # Per head, PE-heavy j=3 slots interleave with light
                        # j=1 slots (and j=2 with j=0) so per-slot PE/ACT
                        # balance is uniform and the PE keeps a backlog (its
                        # clock droops off max p-state whenever it idles).
                        # Pairing (3,1) then (2,0) keeps the po/pr pool rings
                        # deadlock-free: both live j's retire before the next
                        # two allocate.
                        def head_slots(h):
                            out = []
                            for ja, jb in ((3, 1), (2, 0)):
                                a = [(h, ja, p) for p in range(2 * ja + 2)]
                                b = [(h, jb, p) for p in range(2 * jb + 2)]
                                while a or b:
                                    if a:
                                        out.append(a.pop(0))
                                    if a:
                                        out.append(a.pop(0))
                                    if b:
                                        out.append(b.pop(0))
                            return out

                        slots = [s for h in range(HPG) for s in head_slots(h)]

                        def trim(j, i):
                            return P * (i - 4 * j) if i >= 4 * j else 0

                        def emit_S(idx):
                            h, j, p = slots[idx]
                            psS = cps.tile([P, 2, FN], F32, tag="psS")
                            for c in range(2):
                                i = 2 * p + c
                                off = trim(j, i)
                                nc.tensor.matmul(
                                    psS[:, c, off:FN],
                                    qkT[HPG + h][:, i * P:(i + 1) * P],
                                    qkT[h][:, j * FN + off:(j + 1) * FN],
                                    start=True, stop=True,
                                )
                            return psS

                        pos = {}
                        prs_t = {}

                        def emit_PV(k):
                            h, j, p = slots[k]
                            ptt = ptts[k]
                            if p == 0:
                                pos[h, j] = ops.tile(
                                    [P, FN], F32, tag="po", name=f"po{h}_{j}"
                                )
                                prs_t[h, j] = prs.tile(
                                    [1, FN], F32, tag="pr", name=f"pr{h}_{j}"
                                )
                            po = pos[h, j]
                            pr = prs_t[h, j]
                            nk = 2 * j + 2
                            for c in range(2):
                                i = 2 * p + c
                                off = trim(j, i)
                                nc.tensor.matmul(
                                    po[:, off:FN], vn[:, h, i, :],
                                    ptt[:, c, off:FN],
                                    start=(p == 0 and c == 0),
                                    stop=(p == nk - 1 and c == 1),
                                    skip_group_check=True,
                                )
                                # softmax denominator: column sums of P^T
                                # accumulated in PSUM alongside PV
                                nc.tensor.matmul(
                                    pr[:, off:FN], ones_bf[:],
                                    ptt[:, c, off:FN],
                                    start=(p == 0 and c == 0),
                                    stop=(p == nk - 1 and c == 1),
                                    skip_group_check=True,
                                )
                            if p != nk - 1:
                                return
                            rinv = rr_pool.tile([1, FN], F32, tag="rinv")
                            nc.vector.reciprocal_approx_fast(rinv[:], pr[:])
                            # broadcast 1/r across partitions via a DMA
                            # bounce (keeps all engines out of it)
                            rd = dram_pool.tile([1, FN], F32, name=f"rv{h}_{j}")
                            nc.sync.dma_start(rd[:], rinv[:])
                            rb = rr_pool.tile([P, FN], F32, tag="rb")
                            nc.sync.dma_start(
                                rb[:], rd[0:1, :].to_broadcast((P, FN))
                            )
                            nc.vector.tensor_mul(
                                oT[:, h, j * FN:(j + 1) * FN], po[:], rb[:]
                            )

                        # PV runs one slot behind S/exp so the PE never
                        # waits on the exp->mask chain of the current slot
                        LOOK = 1
                        psq = [emit_S(i) for i in range(LOOK)]
                        ptts = {}
                        for k, (h, j, p) in enumerate(slots):
                            if k + LOOK < len(slots):
                                psq.append(emit_S(k + LOOK))
                            psS_cur = psq.pop(0)
                            ptt = pt_pool.tile([P, 2, FN], BF16, tag="ptt")
                            # both chunks trimmed to the c=0 chunk's window
                            # (c=1's extra 128 cols are stale, never read)
                            off0 = trim(j, 2 * p)
                            nc.scalar.activation(
                                ptt[:, :, off0:FN], psS_cur[:, :, off0:FN],
                                mybir.ActivationFunctionType.Exp, scale=SCALE,
                            )
                            if p >= 2 * j:
                                # mask the leading 128x128 triangle blocks
                                for c in range(2):
                                    off = trim(j, 2 * p + c)
                                    nc.vector.tensor_mul(
                                        ptt[:, c, off:off + P],
                                        ptt[:, c, off:off + P],
                                        masks_sb[:, 0:P],
                                    )
                            ptts[k] = ptt
                            if k >= 1:
                                emit_PV(k - 1)
                                del ptts[k - 1]
                        emit_PV(len(slots) - 1)

                    # ---------- Phase D: out = O @ w_proj (partial) ----------
                    with (
                        tc.tile_pool(name="dps", bufs=4, space="PSUM") as dps,
                        tc.tile_pool(name="dstage", bufs=4) as dstage,
                    ):
                        for tm in range(TM):
                            for n in range(C // FN):
                                ps = dps.tile([P, FN], F32)
                                for hh in range(HPG):
                                    nc.tensor.matmul(
                                        ps[:], oT[:, hh, tm * P:(tm + 1) * P],
                                        wpt[:, hh, n * FN:(n + 1) * FN],
                                        start=(hh == 0), stop=(hh == HPG - 1),
                                    )
                                st = dstage.tile([P, FN], F32)
                                if tm == TM - 1:
                                    # drain the tail fast: halves on the
                                    # quicker DVE path, DMA starts earlier
                                    for hf in range(2):
                                        sl = slice(hf * (FN // 2),
                                                   (hf + 1) * (FN // 2))
                                        nc.vector.tensor_copy(st[:, sl], ps[:, sl])
                                        nc.sync.dma_start(
                                            out[tm * P:(tm + 1) * P,
                                                n * FN + hf * (FN // 2):
                                                n * FN + (hf + 1) * (FN // 2)],
                                            st[:, sl],
                                        )
                                    continue
                                if n % 2 == 0:
                                    nc.vector.tensor_copy(st[:], ps[:])
                                else:
                                    nc.scalar.copy(st[:], ps[:])
                                nc.sync.dma_start(
                                    out[tm * P:(tm + 1) * P, n * FN:(n + 1) * FN],
                                    st[:],
                                )
    nc.compile()
    return nc


def ps_dest_vn(vn, nv, tm):
    # psum [128, 512] covers 4 heads' d-columns for this (nv, tm)
    return vn[:, 4 * nv:4 * (nv + 1), tm, :]


def _build_masks() -> np.ndarray:
    """Multiplicative 0/1 triangle for the 128x128 diagonal block of
    P^T = exp(S^T): valid iff q >= k, i.e. col >= row."""
    import ml_dtypes

    rr = np.arange(P)[:, None]
    cc = np.arange(P)[None, :]
    return np.where(cc >= rr, 1.0, 0.0).astype(ml_dtypes.bfloat16)


_CACHE: dict = {}


def _get_nc() -> bass.Bass:
    if "nc" not in _CACHE:
        _CACHE["nc"] = build_nc()
    return _CACHE["nc"]


def _make_in_maps(x, w_qkv, w_proj):
    import ml_dtypes

    bf16 = ml_dtypes.bfloat16
    x = np.asarray(x, dtype=np.float32)
    w_qkv = np.asarray(w_qkv, dtype=np.float32)
    w_proj = np.asarray(w_proj, dtype=np.float32)
    masks = _build_masks()
    in_maps = []
    for core in range(N_CORES):
        b, g = divmod(core, G)
        wq = w_qkv[:, DG * g:DG * (g + 1)]
        wk = w_qkv[:, C + DG * g:C + DG * (g + 1)]
        wvs = w_qkv[:, 2 * C + DG * g:2 * C + DG * (g + 1)]
        w_qk = np.concatenate([wq, wk], axis=1)  # [C, 2048]
        # [ci*128+p, m*128+col] -> [m, p, ci*128+col]
        wqk_perm = np.ascontiguousarray(
            w_qk.reshape(CI, P, MQK, P).transpose(2, 1, 0, 3).reshape(MQK, P, CI * P)
        ).astype(bf16)
        # [ci*128+p, nv*512+vc] -> [nv, p, ci*512+vc]
        wv_perm = np.ascontiguousarray(
            wvs.reshape(CI, P, NV, FN).transpose(2, 1, 0, 3).reshape(NV, P, CI * FN)
        ).astype(bf16)
        wpg = w_proj[DG * g:DG * (g + 1), :]  # [1024, 2048]
        wp_perm = np.ascontiguousarray(
            wpg.reshape(HPG, P, C).transpose(1, 0, 2)  # [p, hh, n]
        ).astype(bf16)
        in_maps.append({
            "xt": np.ascontiguousarray(x[b].T).astype(bf16),
            "wqk": wqk_perm,
            "wv": wv_perm,
            "wp": wp_perm,
            "masks": masks,
        })
    return in_maps


def run_spmd(x, w_qkv, w_proj, trace: bool = False):
    """Returns (out [B,T,C] fp32, BassKernelResults)."""
    in_maps = _make_in_maps(x, w_qkv, w_proj)
    kr = run_bass_kernel_spmd(_get_nc(), in_maps, list(range(N_CORES)), trace=trace)
    res = kr.results
    out = np.empty((B, T, C), dtype=np.float32)
    for b in range(B):
        out[b] = res[G * b]["out"] + res[G * b + 1]["out"]
    return out, kr


def kernel(x, w_qkv, w_proj) -> np.ndarray:
    out, _ = run_spmd(x, w_qkv, w_proj, trace=False)
    return out


# revision 62
# speedup vs baseline: 1.0380x; 1.0380x over previous
"""Causal self-attention on 8 TRN2 NeuronCores.

Sharding: 8 cores = 4 batches x 2 head-groups (data parallel on B,
tensor parallel on heads). Core (b, g) computes batch b, heads
8g..8g+7 end-to-end (qkv slice -> causal attention -> partial
projection); the host sums the two per-batch partials.

All-bf16 dataflow: q/k/v/O stay SBUF-resident (no DRAM round-trips),
matmuls run bf16 (same PE rate as fp32r, but FWL halves the stationary
load and all DMA/SBUF footprints halve). Attention runs as one flat
software pipeline across all heads (pair-of-k-chunks slots; PV/r one
slot behind S/exp so the PE never waits on the exp chain). Diagonal
chunks are trimmed to their valid q-suffix, with only the 128x128
leading triangle masked (multiplicative 0/1 on DVE after exp). The
softmax denominator is PSUM-accumulated by ones-matmuls riding with
PV; 1/r uses the fast DVE reciprocal and a DMA-bounce broadcast.

Self-contained: hardcodes B=4, T=2048, C=2048, H=16, HD=128.
"""

import numpy as np

import concourse.bass as bass
import concourse.mybir as mybir
import concourse.tile as tile
from concourse import bacc
from concourse.bass_utils import run_bass_kernel_spmd

B, T, C, H = 4, 2048, 2048, 16
HD = 128          # head dim
G = 2             # head groups (tensor parallel)
HPG = H // G      # 8 heads per core
DG = HPG * HD     # 1024 = per-core concat head dim
N_CORES = 8
SCALE = float(HD) ** -0.5
NEG = -1.0e9      # additive causal mask value

F32 = mybir.dt.float32
F32R = mybir.dt.float32r
BF16 = mybir.dt.bfloat16

P = 128           # partitions
FN = 512          # moving free-dim per matmul (one PSUM bank of fp32)
CI = C // P       # 16 contraction chunks over C
TM = T // P       # 16 t chunks of 128
TN = T // FN      # 4 t chunks of 512
MQK = 2 * DG // P # 16 qk^T row chunks
NV = DG // FN     # 2 v column chunks of 512
HC = CI // 2      # wv streamed in half-ci tiles


def build_nc() -> bass.Bass:
    nc = bacc.Bacc()
    # xt = x.T (host pre-transposed, bf16). wqk = [wq|wk] cols for this
    # core's heads, host-permuted to [m, p, ci*128+col]. wv = v cols,
    # host-permuted to [nv, p, ci*512+vc]. wp = w_proj rows, host-
    # permuted to [p, hh, n]. All weights bf16.
    xt = nc.declare_dram_parameter("xt", [C, T], BF16, isOutput=False)
    wqk = nc.declare_dram_parameter("wqk", [MQK, P, CI * P], BF16, isOutput=False)
    wv = nc.declare_dram_parameter("wv", [NV, P, CI * FN], BF16, isOutput=False)
    wp = nc.declare_dram_parameter("wp", [P, HPG, C], BF16, isOutput=False)
    masks = nc.declare_dram_parameter("masks", [P, P], BF16, isOutput=False)
    out = nc.declare_dram_parameter("out", [T, C], F32, isOutput=True)

    with tile.TileContext(nc) as tc:
        with (
            tc.tile_pool(name="consts", bufs=1) as consts,
            tc.tile_pool(name="dram", bufs=1, space="DRAM") as dram_pool,
        ):
            ones_f = consts.tile([P, 1], F32)
            nc.gpsimd.memset(ones_f[:], 1.0)
            ones_bf = consts.tile([P, 1], BF16)
            nc.scalar.copy(ones_bf[:], ones_f[:])
            masks_sb = consts.tile([P, P], BF16)  # 0/1 diagonal triangle
            nc.sync.dma_start(masks_sb[:], masks[:, :])
            # warm the exp activation table while ACT is idle
            warm = consts.tile([1, 1], F32)
            nc.scalar.activation(
                warm[:], ones_f[0:1, :], mybir.ActivationFunctionType.Exp
            )

            with (
                tc.tile_pool(name="qkT", bufs=1) as qkT_pool,
                tc.tile_pool(name="vn", bufs=1) as vn_pool,
            ):
                # qk^T resident: m 0..7 = q^T per head, 8..15 = k^T per head
                qkT = [qkT_pool.tile([P, T], BF16, name=f"qkT{m}") for m in range(MQK)]
                # V natural chunks: [t-within-chunk, head, tm, d]
                vn = vn_pool.tile([P, HPG, TM, HD], BF16)

                with tc.tile_pool(name="xT", bufs=1) as xT_pool:
                    xT = xT_pool.tile([P, CI, T], BF16)  # x^T resident 64KB/part

                    # ---------- Phase A: qk^T = wqk.T @ x.T ----------
                    # nt-outer: each nt sweep needs only its 2MB of x^T, so
                    # the PE starts a few us in. wq is re-streamed per nt
                    # (4x wqk traffic, all hidden under compute).
                    QC = 4  # ci per wv tile
                    with (
                        tc.tile_pool(name="wq", bufs=3) as wq_pool,
                        tc.tile_pool(name="aps", bufs=4, space="PSUM") as apsum,
                        tc.tile_pool(name="wvp", bufs=1) as wv_pool,
                        tc.tile_pool(name="bps", bufs=4, space="PSUM") as bpsum,
                    ):
                        wq_pre = []
                        for m in range(2):  # first wq tiles ahead of the
                            w = wq_pool.tile([P, CI, P], BF16, tag="wq")  # x DMAs
                            nc.sync.dma_start(
                                w[:],
                                wqk[m, :, :].rearrange("p (ci n) -> p ci n", ci=CI),
                            )
                            wq_pre.append(w)
                        wvt = {}
                        for nt in range(TN):
                            t0, t1 = nt * FN, (nt + 1) * FN
                            for ci in range(CI):
                                nc.sync.dma_start(
                                    xT[:, ci, t0:t1], xt[ci * P:(ci + 1) * P, t0:t1]
                                )
                            for m in range(MQK):
                                if nt == 0 and m < 2:
                                    wq = wq_pre[m]
                                else:
                                    wq = wq_pool.tile([P, CI, P], BF16, tag="wq")
                                    nc.sync.dma_start(
                                        wq[:],
                                        wqk[m, :, :]
                                        .rearrange("p (ci n) -> p ci n", ci=CI),
                                    )
                                ps = apsum.tile([P, FN], F32)
                                for ci in range(CI):
                                    nc.tensor.matmul(
                                        ps[:], wq[:, ci, :], xT[:, ci, t0:t1],
                                        start=(ci == 0), stop=(ci == CI - 1),
                                    )
                                if m % 2 == 0:
                                    nc.vector.tensor_copy(qkT[m][:, t0:t1], ps[:])
                                else:
                                    nc.scalar.copy(qkT[m][:, t0:t1], ps[:])
                            if nt == 0:
                                # wv resident loads ride behind nt 0's data,
                                # arriving long before phase B
                                for nv in range(NV):
                                    for q in range(CI // QC):
                                        t = wv_pool.tile(
                                            [P, QC, FN], BF16, name=f"wvt{nv}_{q}"
                                        )
                                        nc.sync.dma_start(
                                            t[:],
                                            wv[nv, :, q * QC * FN:(q + 1) * QC * FN]
                                            .rearrange("p (ci n) -> p ci n", ci=QC),
                                        )
                                        wvt[nv, q] = t

                        # ---------- Phase B: V = x @ wv ----------
                        for nv in range(NV):
                            for tm in range(TM):
                                ps = bpsum.tile([P, FN], F32)
                                for ci in range(CI):
                                    nc.tensor.matmul(
                                        ps[:], xT[:, ci, tm * P:(tm + 1) * P],
                                        wvt[nv, ci // QC][:, ci % QC, :],
                                        start=(ci == 0), stop=(ci == CI - 1),
                                    )
                                # DVE-only near the end so ACT is free for
                                # phase C's first exps
                                if tm % 2 == 0 or (nv == NV - 1 and tm >= TM - 4):
                                    nc.vector.tensor_copy(ps_dest_vn(vn, nv, tm), ps[:])
                                else:
                                    nc.scalar.copy(ps_dest_vn(vn, nv, tm), ps[:])

                # xT freed; wpt + oT live in its space through C..D
                with (
                    tc.tile_pool(name="wp", bufs=1) as wp_pool,
                    tc.tile_pool(name="oTp", bufs=1) as oT_pool,
                ):
                    wpt = wp_pool.tile([P, HPG, C], BF16)
                    for hh in range(HPG):
                        nc.sync.dma_start(wpt[:, hh, :], wp[:, hh, :])
                    oT = oT_pool.tile([P, HPG, T], BF16)  # attention out^T

                    # ---------- Phase C: causal attention per head ----------
                    with (
                        tc.tile_pool(name="cps", bufs=2, space="PSUM") as cps,
                        tc.tile_pool(name="ops", bufs=2, space="PSUM") as ops,
                        tc.tile_pool(name="prs", bufs=2, space="PSUM") as prs,
                        tc.tile_pool(name="pt", bufs=6) as pt_pool,
                        tc.tile_pool(name="rr", bufs=4) as rr_pool,
                    ):
                        # one flat pipeline over all heads: slot = (h, j, p)
                        # with 2 k-chunks per slot; no per-head drain.
                        # Diagonal chunks (i >= 4j) are trimmed: the fully-
                        # invalid q-prefix of width 128*(i-4j) is skipped in
                        # S/PV/r (the stale ptt there is never read), and only
                        # the 128x128 leading triangle needs a 0/1 mask.
                        # Per head, PE-heavy j=3 slots interleave with light
                        # j=1 slots (and j=2 with j=0) so per-slot PE/ACT
                        # balance is uniform and the PE keeps a backlog (its
                        # clock droops off max p-state whenever it idles).
                        # Pairing (3,1) then (2,0) keeps the po/pr pool rings
                        # deadlock-free: both live j's retire before the next
                        # two allocate.
                        def head_slots(h):
                            out = []
                            for ja, jb in ((3, 1), (2, 0)):
                                a = [(h, ja, p) for p in range(2 * ja + 2)]
                                b = [(h, jb, p) for p in range(2 * jb + 2)]
                                while a or b:
                                    if a:
                                        out.append(a.pop(0))
                                    if a:
                                        out.append(a.pop(0))
                                    if b:
                                        out.append(b.pop(0))
                            return out

                        slots = [s for h in range(HPG) for s in head_slots(h)]

                        def trim(j, i):
                            return P * (i - 4 * j) if i >= 4 * j else 0

                        def emit_S(idx):
                            h, j, p = slots[idx]
                            psS = cps.tile([P, 2, FN], F32, tag="psS")
                            for c in range(2):
                                i = 2 * p + c
                                off = trim(j, i)
                                nc.tensor.matmul(
                                    psS[:, c, off:FN],
                                    qkT[HPG + h][:, i * P:(i + 1) * P],
                                    qkT[h][:, j * FN + off:(j + 1) * FN],
                                    start=True, stop=True,
                                )
                            return psS

                        pos = {}
                        prs_t = {}

                        def emit_PV(k):
                            h, j, p = slots[k]
                            ptt = ptts[k]
                            if p == 0:
                                pos[h, j] = ops.tile(
                                    [P, FN], F32, tag="po", name=f"po{h}_{j}"
                                )
                                prs_t[h, j] = prs.tile(
                                    [1, FN], F32, tag="pr", name=f"pr{h}_{j}"
                                )
                            po = pos[h, j]
                            pr = prs_t[h, j]
                            nk = 2 * j + 2
                            for c in range(2):
                                i = 2 * p + c
                                off = trim(j, i)
                                nc.tensor.matmul(
                                    po[:, off:FN], vn[:, h, i, :],
                                    ptt[:, c, off:FN],
                                    start=(p == 0 and c == 0),
                                    stop=(p == nk - 1 and c == 1),
                                    skip_group_check=True,
                                )
                                # softmax denominator: column sums of P^T
                                # accumulated in PSUM alongside PV
                                nc.tensor.matmul(
                                    pr[:, off:FN], ones_bf[:],
                                    ptt[:, c, off:FN],
                                    start=(p == 0 and c == 0),
                                    stop=(p == nk - 1 and c == 1),
                                    skip_group_check=True,
                                )
                            if p != nk - 1:
                                return
                            rinv = rr_pool.tile([1, FN], F32, tag="rinv")
                            nc.vector.reciprocal_approx_fast(rinv[:], pr[:])
                            # broadcast 1/r across partitions via a DMA
                            # bounce (keeps all engines out of it)
                            rd = dram_pool.tile([1, FN], F32, name=f"rv{h}_{j}")
                            nc.sync.dma_start(rd[:], rinv[:])
                            rb = rr_pool.tile([P, FN], F32, tag="rb")
                            nc.sync.dma_start(
                                rb[:], rd[0:1, :].to_broadcast((P, FN))
                            )
                            nc.vector.tensor_mul(
                                oT[:, h, j * FN:(j + 1) * FN], po[:], rb[:]
                            )

                        # PV runs one slot behind S/exp so the PE never
                        # waits on the exp->mask chain of the current slot
                        LOOK = 1
                        psq = [emit_S(i) for i in range(LOOK)]
                        ptts = {}
                        for k, (h, j, p) in enumerate(slots):
                            if k + LOOK < len(slots):
                                psq.append(emit_S(k + LOOK))
                            psS_cur = psq.pop(0)
                            ptt = pt_pool.tile([P, 2, FN], BF16, tag="ptt")
                            # both chunks trimmed to the c=0 chunk's window
                            # (c=1's extra 128 cols are stale, never read)
                            off0 = trim(j, 2 * p)
                            nc.scalar.activation(
                                ptt[:, :, off0:FN], psS_cur[:, :, off0:FN],
                                mybir.ActivationFunctionType.Exp, scale=SCALE,
                            )
                            if p >= 2 * j:
                                # mask the leading 128x128 triangle blocks
                                for c in range(2):
                                    off = trim(j, 2 * p + c)
                                    nc.vector.tensor_mul(
                                        ptt[:, c, off:off + P],
                                        ptt[:, c, off:off + P],
                                        masks_sb[:, 0:P],
                                    )
                            ptts[k] = ptt
                            if k >= 1:
                                emit_PV(k - 1)
                                del ptts[k - 1]
                        emit_PV(len(slots) - 1)

                    # ---------- Phase D: out = O @ w_proj (partial) ----------
                    with (
                        tc.tile_pool(name="dps", bufs=4, space="PSUM") as dps,
                        tc.tile_pool(name="dstage", bufs=4) as dstage,
                    ):
                        for tm in range(TM):
                            for n in range(C // FN):
                                ps = dps.tile([P, FN], F32)
                                for hh in range(HPG):
                                    nc.tensor.matmul(
                                        ps[:], oT[:, hh, tm * P:(tm + 1) * P],
                                        wpt[:, hh, n * FN:(n + 1) * FN],
                                        start=(hh == 0), stop=(hh == HPG - 1),
                                    )
                                st = dstage.tile([P, FN], F32)
                                if n % 2 == 0:
                                    nc.vector.tensor_copy(st[:], ps[:])
                                else:
                                    nc.scalar.copy(st[:], ps[:])
                                nc.sync.dma_start(
                                    out[tm * P:(tm + 1) * P, n * FN:(n + 1) * FN],
                                    st[:],
                                )
    nc.compile()
    return nc


def ps_dest_vn(vn, nv, tm):
    # psum [128, 512] covers 4 heads' d-columns for this (nv, tm)
    return vn[:, 4 * nv:4 * (nv + 1), tm, :]


def _build_masks() -> np.ndarray:
    """Multiplicative 0/1 triangle for the 128x128 diagonal block of
    P^T = exp(S^T): valid iff q >= k, i.e. col >= row."""
    import ml_dtypes

    rr = np.arange(P)[:, None]
    cc = np.arange(P)[None, :]
    return np.where(cc >= rr, 1.0, 0.0).astype(ml_dtypes.bfloat16)


_CACHE: dict = {}


def _get_nc() -> bass.Bass:
    if "nc" not in _CACHE:
        _CACHE["nc"] = build_nc()
    return _CACHE["nc"]


def _make_in_maps(x, w_qkv, w_proj):
    import ml_dtypes

    bf16 = ml_dtypes.bfloat16
    x = np.asarray(x, dtype=np.float32)
    w_qkv = np.asarray(w_qkv, dtype=np.float32)
    w_proj = np.asarray(w_proj, dtype=np.float32)
    masks = _build_masks()
    in_maps = []
    for core in range(N_CORES):
        b, g = divmod(core, G)
        wq = w_qkv[:, DG * g:DG * (g + 1)]
        wk = w_qkv[:, C + DG * g:C + DG * (g + 1)]
        wvs = w_qkv[:, 2 * C + DG * g:2 * C + DG * (g + 1)]
        w_qk = np.concatenate([wq, wk], axis=1)  # [C, 2048]
        # [ci*128+p, m*128+col] -> [m, p, ci*128+col]
        wqk_perm = np.ascontiguousarray(
            w_qk.reshape(CI, P, MQK, P).transpose(2, 1, 0, 3).reshape(MQK, P, CI * P)
        ).astype(bf16)
        # [ci*128+p, nv*512+vc] -> [nv, p, ci*512+vc]
        wv_perm = np.ascontiguousarray(
            wvs.reshape(CI, P, NV, FN).transpose(2, 1, 0, 3).reshape(NV, P, CI * FN)
        ).astype(bf16)
        wpg = w_proj[DG * g:DG * (g + 1), :]  # [1024, 2048]
        wp_perm = np.ascontiguousarray(
            wpg.reshape(HPG, P, C).transpose(1, 0, 2)  # [p, hh, n]
        ).astype(bf16)
        in_maps.append({
            "xt": np.ascontiguousarray(x[b].T).astype(bf16),
            "wqk": wqk_perm,
            "wv": wv_perm,
            "wp": wp_perm,
            "masks": masks,
        })
    return in_maps


def run_spmd(x, w_qkv, w_proj, trace: bool = False):
    """Returns (out [B,T,C] fp32, BassKernelResults)."""
    in_maps = _make_in_maps(x, w_qkv, w_proj)
    kr = run_bass_kernel_spmd(_get_nc(), in_maps, list(range(N_CORES)), trace=trace)
    res = kr.results
    out = np.empty((B, T, C), dtype=np.float32)
    for b in range(B):
        out[b] = res[G * b]["out"] + res[G * b + 1]["out"]
    return out, kr


def kernel(x, w_qkv, w_proj) -> np.ndarray:
    out, _ = run_spmd(x, w_qkv, w_proj, trace=False)
    return out
